# revision 22
# baseline (speedup 1.0000x reference)
"""MicroTransformer (B=16,S=512,V=8000,D=5,F=20,L=2) on 8 trn2 NeuronCores.

Sharding: pure data parallel over batch (2 batch elements per core).
All parameters replicated. Whole transformer body + logits matmul run on
device; host only does input prep (embedding row gather, positional
encoding constant, weight layout transforms) and the final concat.

Per-core device program (Bass/Tile, fully unrolled):
  state hT [6, 1024] f32: rows 0-4 = h^T for batch0|batch1 (cols 0-511 /
  512-1023), row 5 = ones (bias row for augmented matmuls).
  per layer, per batch:
    qT/kT      = Wq_aug/Wk_aug [6,5] x hT          (PSUM -> SBUF)
    V [k,d]    = hT-chunk [6,128] x Wv_aug [6,5]   (natural [s,d] layout)
    scores     = qT-chunk [5,128] x kT [5,512]     -> PSUM [128,512]
    ttr        = (scores + mask)*(-1/sqrt(5)), rowmax via min-accum
    exp        = ACT Exp(scale=-1, bias=-max), rowsum via accum_out
    attnT      = matmul(attn-block, diag(1/Z))     (transpose + normalize)
    ctxT       = V-chunk [128,5] x attnT [128,128] (PSUM accum over k)
    proj/ffn   = augmented matmuls; LayerNorm via matmul stats + bcast
  logits: hfin [6,128] x fcw_aug [6,500] -> PSUM -> copy (DVE/ACT split)
          -> SBUF stage [128,2000] -> 1MB DMA stores.
"""

import math

import numpy as np

import concourse.bacc as bacc
import concourse.bass as bass
import concourse.mybir as mybir
import concourse.tile as tile
from concourse.bass_utils import run_bass_kernel_spmd

F32 = mybir.dt.float32
F32R = mybir.dt.float32r
ALU = mybir.AluOpType
ACTF = mybir.ActivationFunctionType


def _r(ap):
    """float32r view: 4x PE throughput at N>=256, ~bf16-pair precision."""
    return ap.bitcast(F32R)

B, S, V, D, F, L = 16, 512, 8000, 5, 20, 2
EPS = 1e-5
NCORES = 8
BPC = B // NCORES  # batches per core = 2
SQRT_D = math.sqrt(float(D))
SCALE = 1.0 / SQRT_D
NEG = -1.0e30
QC = S // 128  # 4 q/k chunks of 128
VCH = 500      # vocab chunk per matmul (<=512 psum bank)
NVC = V // VCH                 # 16
VGRP = 4                       # vocab chunks per DMA stage
NVG = NVC // VGRP              # 4 stages of 2000 vocab

_CACHED = None  # (nc, names)


def _build_program():
    nc = bacc.Bacc("TRN2", target_bir_lowering=False, debug=False,
                   num_devices=NCORES)

    # ---- DRAM I/O ----
    d_h0 = nc.dram_tensor("h0", [D + 1, BPC * S], F32, kind="ExternalInput")
    d_wq = nc.dram_tensor("wq", [L, D + 1, D], F32, kind="ExternalInput")
    d_wk = nc.dram_tensor("wk", [L, D + 1, D], F32, kind="ExternalInput")
    d_wv = nc.dram_tensor("wv", [L, D + 1, D], F32, kind="ExternalInput")
    d_wo = nc.dram_tensor("wo", [L, D + 1, D], F32, kind="ExternalInput")
    d_w1 = nc.dram_tensor("w1", [L, D + 1, F], F32, kind="ExternalInput")
    d_w2 = nc.dram_tensor("w2", [L, F + 1, D], F32, kind="ExternalInput")
    d_gcol = nc.dram_tensor("gcol", [1, 2 * L * D], F32R, kind="ExternalInput")
    d_lnb = nc.dram_tensor("lnb", [D, 2 * L], F32, kind="ExternalInput")
    d_c02 = nc.dram_tensor("c02", [D, 1], F32R, kind="ExternalInput")
    d_mask = nc.dram_tensor("mask", [128, QC, S], F32, kind="ExternalInput")
    d_eye = nc.dram_tensor("eye", [128, 128], F32, kind="ExternalInput")
    d_fcw = nc.dram_tensor("fcw", [D + 1, V], F32R, kind="ExternalInput")
    d_ones = nc.dram_tensor("ones", [1, S], F32, kind="ExternalInput")
    d_onesr = nc.dram_tensor("onesr", [1, S], F32R, kind="ExternalInput")
    d_out = nc.dram_tensor("out", [BPC, S, V], F32, kind="ExternalOutput")

    copy_ctr = [0]

    from contextlib import ExitStack
    with tile.TileContext(nc) as tc, ExitStack() as es, \
            nc.allow_low_precision(reason="float32r rounding is intended"):
        cst = es.enter_context(tc.tile_pool(name="cst", bufs=1))
        wrk = es.enter_context(tc.tile_pool(name="wrk", bufs=2))
        att = es.enter_context(tc.tile_pool(name="att", bufs=2))
        stg = es.enter_context(tc.tile_pool(name="stg", bufs=3))
        ps_a = es.enter_context(tc.tile_pool(name="ps_a", bufs=4, space="PSUM"))
        ps_s = es.enter_context(tc.tile_pool(name="ps_s", bufs=3, space="PSUM"))
        ps_c = es.enter_context(tc.tile_pool(name="ps_c", bufs=1, space="PSUM"))

        # ---- constants into SBUF ----
        h = cst.tile([D + 1, BPC * S], F32, name="h", tag="h")
        nc.sync.dma_start(h[:], d_h0[:])
        mask = cst.tile([128, QC, S], F32, name="mask", tag="mask")
        nc.sync.dma_start(mask[:], d_mask[:])
        eye = cst.tile([128, 128], F32, name="eye", tag="eye")
        nc.sync.dma_start(eye[:], d_eye[:])
        fcw = cst.tile([D + 1, V], F32R, name="fcw", tag="fcw")
        nc.sync.dma_start(fcw[:], d_fcw[:])
        gcol = cst.tile([1, 2 * L * D], F32R, name="gcol", tag="gcol")
        nc.sync.dma_start(gcol[:], d_gcol[:])
        lnb = cst.tile([D, 2 * L], F32, name="lnb", tag="lnb")
        nc.sync.dma_start(lnb[:], d_lnb[:])
        c02 = cst.tile([D, 1], F32R, name="c02", tag="c02")
        nc.sync.dma_start(c02[:], d_c02[:])
        eps1 = cst.tile([1, 1], F32, name="eps1", tag="eps1")
        nc.vector.memset(eps1[:], EPS)
        wq, wk, wv, wo, w1, w2 = [], [], [], [], [], []
        for l in range(L):
            for lst, dt_, shp, nm in (
                (wq, d_wq, [D + 1, D], "wq"), (wk, d_wk, [D + 1, D], "wk"),
                (wv, d_wv, [D + 1, D], "wv"), (wo, d_wo, [D + 1, D], "wo"),
                (w1, d_w1, [D + 1, F], "w1"), (w2, d_w2, [F + 1, D], "w2"),
            ):
                t = cst.tile(shp, F32, name=f"{nm}{l}", tag=f"{nm}{l}")
                nc.sync.dma_start(t[:], dt_[l])
                lst.append(t)
        hfin = []
        for b in range(BPC):
            t = cst.tile([D + 1, S], F32R, name=f"hfin{b}", tag=f"hfin{b}")
            nc.sync.dma_start(t[D:D + 1, :], d_onesr[:])
            hfin.append(t)

        def layernorm(l, i, b, resid_ap, add_ps, out_ap):
            """out = LN(resid + add) * g + b   (g,b exact via bcast lhsT)."""
            x = wrk.tile([D, S], F32R, name=f"lnx{l}{i}{b}", tag="lnx")
            nc.vector.tensor_add(x[:], resid_ap, add_ps)
            xf = x[:].bitcast(F32)
            xsq = wrk.tile([D, S], F32R, name=f"lnq{l}{i}{b}", tag="lnq")
            nc.vector.tensor_mul(xsq[:], xf, xf)
            s1 = ps_s.tile([1, S], F32, name=f"s1_{l}{i}{b}", tag="ps_s")
            s2 = ps_s.tile([1, S], F32, name=f"s2_{l}{i}{b}", tag="ps_s")
            nc.tensor.matmul(s1[:], c02[:], x[:])      # mean
            nc.tensor.matmul(s2[:], c02[:], xsq[:])    # E[x^2]
            t1 = wrk.tile([1, S], F32, name=f"t1_{l}{i}{b}", tag="lnt1")
            nc.scalar.square(t1[:], s1[:])             # mean^2 (ACT)
            var = wrk.tile([1, S], F32, name=f"lv_{l}{i}{b}", tag="lnvar")
            nc.vector.tensor_sub(var[:], s2[:], t1[:])
            se = wrk.tile([1, S], F32, name=f"se_{l}{i}{b}", tag="lnse")
            nc.scalar.activation(se[:], var[:], ACTF.Sqrt, bias=eps1[:])
            rr = wrk.tile([1, 2 * S], F32R, name=f"rr_{l}{i}{b}", tag="lnrr")
            nc.vector.reciprocal(rr[0:1, 0:S], se[:])
            nc.vector.tensor_mul(rr[0:1, S:2 * S],
                                 rr[0:1, 0:S].bitcast(F32), s1[:])
            gc = gcol[0:1, (l * 2 + i) * D:(l * 2 + i + 1) * D]
            rb = ps_s.tile([D, S], F32, name=f"rb_{l}{i}{b}", tag="ps_s")
            rm = ps_s.tile([D, S], F32, name=f"rm_{l}{i}{b}", tag="ps_s")
            nc.tensor.matmul(rb[:], gc, rr[0:1, 0:S])      # g_d * r_s
            nc.tensor.matmul(rm[:], gc, rr[0:1, S:2 * S])  # g*r*mu
            t2 = wrk.tile([D, S], F32, name=f"t2_{l}{i}{b}", tag="lnt2")
            nc.vector.tensor_mul(t2[:], xf, rb[:])
            bc = lnb[:, (l * 2 + i):(l * 2 + i + 1)]
            # out = (t2 + b_d) - g_d*r*mu
            nc.vector.scalar_tensor_tensor(out_ap, t2[:], bc, rm[:],
                                           op0=ALU.add, op1=ALU.subtract)

        # ---- transformer body ----
        for l in range(L):
            for b in range(BPC):
                hb = h[:, b * S:(b + 1) * S]
                # q/k projections -> [5, 512]
                qps = ps_s.tile([D, S], F32, name=f"qp{l}{b}", tag="ps_s")
                kps = ps_s.tile([D, S], F32, name=f"kp{l}{b}", tag="ps_s")
                nc.tensor.matmul(qps[:], wq[l][:], hb)
                nc.tensor.matmul(kps[:], wk[l][:], hb)
                qsb = wrk.tile([D, S], F32, name=f"qs{l}{b}", tag="qsb")
                ksb = wrk.tile([D, S], F32, name=f"ks{l}{b}", tag="ksb")
                nc.vector.tensor_copy(qsb[:], qps[:])
                nc.vector.tensor_copy(ksb[:], kps[:])
                # V in [k, d] layout
                vps = ps_a.tile([128, QC, D], F32, name=f"vp{l}{b}", tag="ps_a")
                for kc in range(QC):
                    nc.tensor.matmul(vps[:, kc, :],
                                     hb[:, kc * 128:(kc + 1) * 128], wv[l][:])
                vsb = wrk.tile([128, QC, D], F32R, name=f"vs{l}{b}", tag="vsb")
                nc.vector.tensor_copy(vsb[:], vps[:])

                # scores + masked softmax (unnormalized exp + row sums)
                attn = att.tile([128, QC, S], F32R, name=f"at{l}{b}", tag="attn")
                nm = wrk.tile([128, QC], F32, name=f"nm{l}{b}", tag="nm")
                zt = wrk.tile([128, QC], F32, name=f"zt{l}{b}", tag="zt")
                for qc in range(QC):
                    scps = ps_a.tile([128, S], F32, name=f"sc{l}{b}{qc}",
                                     tag="ps_a")
                    nc.tensor.matmul(scps[:], qsb[:, qc * 128:(qc + 1) * 128],
                                     ksb[:])
                    sraw = wrk.tile([128, S], F32, name=f"sr{l}{b}{qc}",
                                    tag="sraw")
                    # sraw = scores * (-1/sqrt(D)) + mask  (masked -> +huge)
                    nc.vector.scalar_tensor_tensor(
                        sraw[:], scps[:], -SCALE, mask[:, qc, :],
                        op0=ALU.mult, op1=ALU.add)
                    # row min of sraw == -(row max of scaled scores)
                    nc.vector.tensor_reduce(
                        nm[:, qc:qc + 1], sraw[:],
                        axis=mybir.AxisListType.X, op=ALU.min)
                    # attn = exp(scaled_scores - max); accum = row sums
                    nc.scalar.activation(attn[:, qc, :], sraw[:], ACTF.Exp,
                                         bias=nm[:, qc:qc + 1], scale=-1.0,
                                         accum_out=zt[:, qc:qc + 1])
                rz = wrk.tile([128, QC], F32, name=f"rz{l}{b}", tag="rz")
                nc.vector.reciprocal(rz[:], zt[:])

                # transpose + normalize via matmul with diag(1/Z)
                diags = []
                for qc in range(QC):
                    dg = wrk.tile([128, 128], F32R, name=f"dg{l}{b}{qc}",
                                  tag=f"dg{qc}")
                    nc.vector.tensor_scalar_mul(dg[:], eye[:],
                                                rz[:, qc:qc + 1])
                    diags.append(dg)
                attnT = att.tile([128, QC, S], F32R, name=f"aT{l}{b}",
                                 tag="attnT")
                for kc in range(QC):
                    atps = ps_a.tile([128, S], F32, name=f"tp{l}{b}{kc}",
                                     tag="ps_a")
                    for qc in range(QC):
                        nc.tensor.matmul(
                            atps[:, qc * 128:(qc + 1) * 128],
                            attn[:, qc, kc * 128:(kc + 1) * 128],
                            diags[qc][:])
                    nc.scalar.copy(attnT[:, kc, :], atps[:])

                # ctx^T [5, 512] accumulated over k chunks
                ctxps = ps_c.tile([D, S], F32, name=f"cx{l}{b}", tag="ps_c")
                for kc in range(QC):
                    nc.tensor.matmul(
                        ctxps[:], vsb[:, kc, :], attnT[:, kc, :],
                        start=(kc == 0), stop=(kc == QC - 1))
                ctxa = wrk.tile([D + 1, S], F32, name=f"ca{l}{b}", tag="ctxa")
                nc.vector.tensor_copy(ctxa[0:D, :], ctxps[:])
                nc.sync.dma_start(ctxa[D:D + 1, :], d_ones[:])
                pps = ps_s.tile([D, S], F32, name=f"pp{l}{b}", tag="ps_s")
                nc.tensor.matmul(pps[:], wo[l][:], ctxa[:])
                layernorm(l, 0, b, hb[0:D, :], pps[:], hb[0:D, :])

                # FFN
                f1ps = ps_s.tile([F, S], F32, name=f"f1{l}{b}", tag="ps_s")
                nc.tensor.matmul(f1ps[:], w1[l][:], hb)
                f1a = wrk.tile([F + 1, S], F32, name=f"fa{l}{b}", tag="f1a")
                nc.scalar.activation(f1a[0:F, :], f1ps[:], ACTF.Relu)
                nc.sync.dma_start(f1a[F:F + 1, :], d_ones[:])
                f2ps = ps_s.tile([D, S], F32, name=f"f2{l}{b}", tag="ps_s")
                nc.tensor.matmul(f2ps[:], w2[l][:], f1a[:])
                out_sl = hfin[b][0:D, :] if l == L - 1 else hb[0:D, :]
                layernorm(l, 1, b, hb[0:D, :], f2ps[:], out_sl)

        # ---- logits ----
        for b in range(BPC):
            for sc in range(QC):
                hsl = hfin[b][:, sc * 128:(sc + 1) * 128]
                for vg in range(NVG):
                    st = stg.tile([128, VGRP * VCH], F32,
                                  name=f"st{b}{sc}{vg}", tag="stage")
                    for vv in range(VGRP):
                        vc = vg * VGRP + vv
                        lp = ps_a.tile([128, VCH], F32,
                                       name=f"lp{b}{sc}{vc}", tag="ps_a")
                        nc.tensor.matmul(lp[:], hsl,
                                         fcw[:, vc * VCH:(vc + 1) * VCH])
                        # split PSUM->SBUF copies across DVE and ACT
                        if copy_ctr[0] % 2 == 0:
                            nc.vector.tensor_copy(
                                st[:, vv * VCH:(vv + 1) * VCH], lp[:])
                        else:
                            nc.scalar.copy(
                                st[:, vv * VCH:(vv + 1) * VCH], lp[:])
                        copy_ctr[0] += 1
                    nc.sync.dma_start(
                        d_out[b, sc * 128:(sc + 1) * 128,
                              vg * VGRP * VCH:(vg + 1) * VGRP * VCH],
                        st[:])

    nc.compile()
    return nc


def _get_program():
    global _CACHED
    if _CACHED is None:
        _CACHED = _build_program()
    return _CACHED


def _pos_encoding_np():
    pos = np.arange(B, dtype=np.float32)[:, None]
    div = np.exp(np.arange(0, D, 2, dtype=np.float32)
                 * (-math.log(10000.0) / D))
    pe = np.zeros((B, D), dtype=np.float32)
    pe[:, 0::2] = np.sin(pos * div)
    pe[:, 1::2] = np.cos(pos * div[:-1])
    return pe


def host_inputs(x, emb, in_proj_w, in_proj_b, out_proj_w, out_proj_b,
                ln1_g, ln1_b, ln2_g, ln2_b, ff1_w, ff1_b, ff2_w, ff2_b,
                fc_w, fc_b):
    """Build the per-core input maps (host-side prep only)."""
    x = np.asarray(x).astype(np.int64)
    emb = np.asarray(emb, dtype=np.float32)
    f32 = lambda a: np.ascontiguousarray(np.asarray(a, dtype=np.float32))
    in_proj_w, in_proj_b = f32(in_proj_w), f32(in_proj_b)
    out_proj_w, out_proj_b = f32(out_proj_w), f32(out_proj_b)
    ff1_w, ff1_b, ff2_w, ff2_b = f32(ff1_w), f32(ff1_b), f32(ff2_w), f32(ff2_b)
    ln1_g, ln1_b, ln2_g, ln2_b = f32(ln1_g), f32(ln1_b), f32(ln2_g), f32(ln2_b)
    fc_w, fc_b = f32(fc_w), f32(fc_b)

    h0 = emb[x] * np.float32(SQRT_D)          # [B, S, D]
    h0 = h0 + _pos_encoding_np()[:, None, :]  # pe-by-batch-index (faithful)
    h0t = np.transpose(h0, (0, 2, 1))         # [B, D, S]

    def aug(wT, bias):  # [K, M] + bias row
        return np.ascontiguousarray(
            np.concatenate([wT, bias[None, :]], axis=0).astype(np.float32))

    wqs = np.stack([aug(in_proj_w[l][0:D].T, in_proj_b[l][0:D])
                    for l in range(L)])
    wks = np.stack([aug(in_proj_w[l][D:2 * D].T, in_proj_b[l][D:2 * D])
                    for l in range(L)])
    wvs = np.stack([aug(in_proj_w[l][2 * D:3 * D].T, in_proj_b[l][2 * D:3 * D])
                    for l in range(L)])
    wos = np.stack([aug(out_proj_w[l].T, out_proj_b[l]) for l in range(L)])
    w1s = np.stack([aug(ff1_w[l].T, ff1_b[l]) for l in range(L)])
    w2s = np.stack([aug(ff2_w[l].T, ff2_b[l]) for l in range(L)])

    gcol = np.zeros((1, 2 * L * D), np.float32)
    lnbm = np.zeros((D, 2 * L), np.float32)
    for l in range(L):
        gcol[0, (2 * l) * D:(2 * l + 1) * D] = ln1_g[l]
        gcol[0, (2 * l + 1) * D:(2 * l + 2) * D] = ln2_g[l]
        lnbm[:, 2 * l] = ln1_b[l]
        lnbm[:, 2 * l + 1] = ln2_b[l]
    c02 = np.full((D, 1), 1.0 / D, np.float32)

    kidx = np.arange(S, dtype=np.int64)
    maskf = np.zeros((128, QC, S), np.float32)
    maskval = np.float32(SCALE * 1.0e30)  # positive: sraw is negated scores
    for qc in range(QC):
        qpos = qc * 128 + np.arange(128)[:, None]
        maskf[:, qc, :] = np.where(kidx[None, :] <= qpos, 0.0, maskval)
    eye = np.eye(128, dtype=np.float32)
    fcw = np.ascontiguousarray(
        np.concatenate([fc_w.T, fc_b[None, :]], axis=0).astype(np.float32))

    shared = dict(wq=wqs, wk=wks, wv=wvs, wo=wos, w1=w1s, w2=w2s,
                  gcol=gcol, lnb=lnbm, c02=c02, mask=maskf, eye=eye, fcw=fcw,
                  ones=np.ones((1, S), np.float32),
                  onesr=np.ones((1, S), np.float32))
    in_maps = []
    for c in range(NCORES):
        hh = np.ones((D + 1, BPC * S), np.float32)
        for b in range(BPC):
            hh[0:D, b * S:(b + 1) * S] = h0t[c * BPC + b]
        in_maps.append(dict(h0=hh, **shared))
    return in_maps


def run(in_maps, trace=False, **kw):
    nc = _get_program()
    return run_bass_kernel_spmd(nc, in_maps, list(range(NCORES)),
                                trace=trace, **kw)


def kernel(**inputs) -> np.ndarray:
    in_maps = host_inputs(**inputs)
    res = run(in_maps)
    out = np.concatenate([res.results[c]["out"] for c in range(NCORES)],
                         axis=0)
    return np.ascontiguousarray(out.astype(np.float32))


if __name__ == "__main__":
    import reference
    ins = {k: np.asarray(v) for k, v in reference.setup_inputs().items()}
    got = kernel(**ins)
    exp = np.asarray(reference.reference(**reference.setup_inputs()))
    err = np.abs(got - exp)
    rel = np.abs(got - exp).max() / (np.abs(exp).max() + 1e-30)
    print("max abs err:", err.max(), "rel:", rel)
